# revision 52
# baseline (speedup 1.0000x reference)
"""Trainium2 Bass kernel: 3-layer spiking NN (DSNN) forward, 50 timesteps.

Strategy (8 NeuronCores, no inter-core communication):
  - Layer-1 output features sharded 8x (256 per core); each core runs the
    full layer-0 dynamics locally (spk0 is a dense input to every shard's
    matmul) and the host sums the K-sharded final layer-2 partials.
  - H0 = inputs @ W0 (time-invariant layer-0 drive) is computed near-exactly
    via three matmul chains (ihi@Whi + ihi@Wlo + ilo@Whi with fp16 hi and
    fp16/bf16 lo splits; ~1e-4 max abs error on H0), directly in
    [feature, batch] layout (W0 chunk as the stationary operand), streamed
    in 2 column groups so the loop's lower-half work starts while the
    upper half is still in DMA (phase-A prefill of the first PREF steps).
  - Layer-0 recurrence on the pre-reset membrane MP via the fused DVE op
    DSNN_LIF_STEP: MP' = beta*(MP*(MP<=1)) + H0.  Spikes via ACT sigmoid
    saturation, written as exact {0,1} float8e4.  All concurrently-used
    regions are separate tiles (dependency tracking is tile-granular).
  - Layer-1 drive: psum += spk0 @ W1 as fp8 DoubleRow matmuls (K=256 per
    matmul, 0.5 cyc/row) over a 2-term e4m3 split of W1 (quantization error
    ~0.16%; spikes are exact in fp8 so only W1 rounding enters).
  - Layer-1 synaptic state S1 lives entirely in PSUM: ACT performs the
    alpha-decay as a scaled psum->psum copy into the next ping-pong bank
    and the matmul group accumulates on top (start=False; note start=True
    zeroes the WHOLE psum bank, so it is only used for the first group).
    The fused LIF op reads S1 straight from PSUM for the membrane update;
    spk1 threshold is split across DVE (2x-rate tensor_scalar) and ACT.
  - Output accumulation A = sum_t c_t * spk1(t) on the PE via c_t-scaled
    identity stationaries; final A @ W2 shard as 2 fp32r matmuls; host sums
    the 8 partial outputs.
Measured (TimelineSim): 181103 ns vs 219440 ns baseline; rel err 1.66e-2.
"""

import numpy as np
import ml_dtypes
from contextlib import ExitStack

import concourse.bacc as bacc
import concourse.bass as bass
import concourse.mybir as mybir
import concourse.tile as tile
from concourse import bass_utils
from concourse import dve_ops as _DOPS
from concourse.dve_spec import Spec, Src0, Src1, C0, One, lower as _dve_lower
from concourse.dve_uop import DveOpSpec as _DveOpSpec

ALPHA = 0.9
BETA = 0.85
T = 50
B = 128                # batch
F0, F1, F3 = 1024, 2048, 512
N_CORES = 8
SH = F1 // N_CORES     # 256 layer-1 features per core
KC0 = F0 // 128        # 8 contraction chunks for H0
NPAIR = F1 // 256      # 8 DoubleRow contraction pairs for layer-1
NG = 2                 # H0 column groups (feature groups of 1024)
GW = F1 // NG          # 1024 features per group
BIG = 1.0e30

f32 = mybir.dt.float32
f32r = mybir.dt.float32r
f16 = mybir.dt.float16
bf16 = mybir.dt.bfloat16
f8 = mybir.dt.float8e4
AL = mybir.AluOpType
AF = mybir.ActivationFunctionType
PM_DR = mybir.MatmulPerfMode.DoubleRow

LIF_SLABS = 2          # layer-0 LIF slabs (1024 cols each)
SIG2_STEPS = 8         # steps that use 2 spike-extraction slabs (prologue overlap)


def _register_lif():
    """Fused LIF step on the pre-reset membrane:
    out = s0 * (in0 * (in0 <= 1)) + in1  (reset-gate, leak, drive)."""
    name = "DSNN_LIF_STEP"
    for op in _DOPS.OPS:
        if op.name == name:
            return op
    body = (Src0 * (Src0 <= One)) * C0 + Src1
    spec = Spec(body=body,
                reference=lambda in0, in1, s0, s1, imm2:
                    ((in0 * (in0 <= 1.0)) * np.float32(s0) + in1).astype(np.float32))
    row = max(_DOPS._SUB_OPCODE_FOR_NAME.values()) + 1
    _DOPS._SUB_OPCODE_FOR_NAME[name] = row
    shas = {}
    for ver in ("v3", "v4"):
        uops = _dve_lower(spec, ver=ver)
        shas[ver] = _DveOpSpec(name=name, opcode=row, uops=uops, rd1_en=True).sha(ver)
    op = _DOPS.DveOp(name, spec, subdim=False, uops_sha=shas)
    _DOPS.OPS.append(op)
    _DOPS.CUSTOM_DVE_SPECS[name] = spec
    return op


LIF = _register_lif()


def _coeffs():
    # m2(T) = sum_{t=1..T} c[t-1] * h2(t)
    c = np.zeros(T, dtype=np.float64)
    for s in range(T):
        tau = s + 1
        c[s] = sum(BETA ** (T - t) * ALPHA ** (t - tau) for t in range(tau, T + 1))
    return c.astype(np.float32)


def _build():
    nc = bacc.Bacc("TRN2", target_bir_lowering=False, debug=False)
    d_IT = nc.dram_tensor("IT", [F0, 2 * B], f16, kind="ExternalInput")     # [infeat, hi|lo batch]
    d_W0H = nc.dram_tensor("W0H", [F0, F1], f16, kind="ExternalInput")
    d_W0L = nc.dram_tensor("W0L", [F0, F1], bf16, kind="ExternalInput")
    d_W1 = nc.dram_tensor("W1AB", [F1, 2 * SH], f8, kind="ExternalInput")   # [feat, A|B cols]
    d_W2 = nc.dram_tensor("W2S", [SH, F3], f32r, kind="ExternalInput")
    d_eye = nc.dram_tensor("EYE", [128, 128], f32, kind="ExternalInput")
    d_out = nc.dram_tensor("OUT", [B, F3], f32, kind="ExternalOutput")

    coefs = _coeffs()

    with tile.TileContext(nc) as tc, ExitStack() as ctx:
        const_pool = ctx.enter_context(tc.tile_pool(name="const", bufs=1))
        state_pool = ctx.enter_context(tc.tile_pool(name="state", bufs=1))
        w0h_pool = ctx.enter_context(tc.tile_pool(name="w0h", bufs=5))
        w0l_pool = ctx.enter_context(tc.tile_pool(name="w0l", bufs=5))
        out_pool = ctx.enter_context(tc.tile_pool(name="outp", bufs=1))
        psH_pool = ctx.enter_context(tc.tile_pool(name="psH", bufs=2, space="PSUM"))
        ps1_pool = ctx.enter_context(tc.tile_pool(name="ps1", bufs=2, space="PSUM"))
        psA_pool = ctx.enter_context(tc.tile_pool(name="psA", bufs=1, space="PSUM"))
        psT_pool = ctx.enter_context(tc.tile_pool(name="psT", bufs=1, space="PSUM"))

        # ---- resident constants (DMAed first; small) --------------------------
        eye = const_pool.tile([128, 128], f32, tag="eye")
        nc.sync.dma_start(eye[:], d_eye.ap())
        itb = const_pool.tile([128, KC0 * 2 * B], f16, tag="itb")
        nc.sync.dma_start(itb[:].rearrange("p (k c) -> p k c", k=KC0),
                          d_IT.ap().rearrange("(k p) c -> p k c", p=128))
        it3 = itb[:].rearrange("p (k c) -> p k c", k=KC0)

        bnbig = const_pool.tile([128, 1], f32, tag="bnbig")
        nc.vector.memset(bnbig[:], -BIG)

        # PE warm-up: the p-state ramp needs ~3us of continuous busy before
        # the PE hits full speed.  A run of dummy eye transposes bridges the
        # gap until the first W0 chunk lands so the H0 matmuls start the
        # stream fully ramped instead of at half rate.
        warm = psT_pool.tile([128, 128], f32, tag="psT", name="warm")
        for _ in range(16):
            nc.tensor.transpose(warm[:], eye[:], eye[:])

        # c_t-scaled identity tiles for the A accumulation (DVE 2x tensor_scalar)
        ctis = const_pool.tile([128, T * 128], f32r, tag="ctis")
        for t in range(T):
            nc.vector.tensor_scalar(ctis[:, t * 128:(t + 1) * 128], eye[:],
                                    float(coefs[t]), None, AL.mult)

        # ---- state ------------------------------------------------------------
        # Dependency tracking is TILE-granular: every region used
        # concurrently by different engines must be its own tile.  Layer-0
        # state is split into column halves (a = features 0:1024,
        # b = 1024:2048) and rotated over NB buffers.
        NB = 3   # state-buffer rotation depth (>2 gives the scheduler slack)
        HW2 = F1 // 2
        H0Th = [state_pool.tile([128, HW2], f32, tag=f"H0T{h}", name=f"H0T{h}")
                for h in range(2)]                             # [feat, batch] drive
        MPh = [[state_pool.tile([128, HW2], f32, tag=f"MP{h}{i}", name=f"MP{h}{i}")
                for i in range(NB)] for h in range(2)]         # layer-0 membrane
        NS0h = [[state_pool.tile([128, HW2], f8, tag=f"NS0{h}{i}", name=f"NS0{h}{i}")
                 for i in range(NB)] for h in range(2)]        # spk0 {0,1} fp8
        M1P = [state_pool.tile([128, SH], f32, tag=f"M1P{i}", name=f"M1P{i}")
               for i in range(NB)]                             # layer-1 membrane (pre-reset)
        NS1 = [state_pool.tile([128, SH], f32r, tag=f"NS1{i}", name=f"NS1{i}")
               for i in range(NB)]
        nc.scalar.memzero(MPh[0][0][:])
        nc.scalar.memzero(MPh[1][0][:])
        nc.scalar.memzero(M1P[0][:])

        # layer-1 synaptic state: PSUM ping-pong banks (S1' = alpha*S1 + spk0@W1)
        ph1 = [ps1_pool.tile([128, SH], f32, tag="ps1", name=f"ph1{i}")
               for i in range(2)]
        A_ps = psA_pool.tile([128, SH], f32, tag="psA")

        # ---- H0 = (inputs @ W0)^T via 3 fp16 chains, streamed in NG groups ----
        phs = [psH_pool.tile([128, GW], f32, tag="psH", name=f"psH{g}")
               for g in range(NG)]
        h0t_upper_copies = []
        for g in range(NG):
            cols = slice(g * GW, (g + 1) * GW)
            for k in range(KC0):
                th = w0h_pool.tile([128, GW], f16, tag="w0h", name=f"w0h_{g}_{k}")
                nc.sync.dma_start(th[:], d_W0H.ap()[k * 128:(k + 1) * 128, cols])
                tl = w0l_pool.tile([128, GW], bf16, tag="w0l", name=f"w0l_{g}_{k}")
                nc.sync.dma_start(tl[:], d_W0L.ap()[k * 128:(k + 1) * 128, cols])
                for jj in range(GW // 128):
                    out = phs[g][:, jj * 128:(jj + 1) * 128]
                    wh = th[:, jj * 128:(jj + 1) * 128]
                    wl = tl[:, jj * 128:(jj + 1) * 128]
                    # start=True zeroes the WHOLE psum bank: only the first
                    # 128-slice of each 512-col bank may set it
                    nc.tensor.matmul(out, wh, it3[:, k, 0:B],
                                     start=(k == 0 and jj % 4 == 0), stop=False,
                                     skip_group_check=True)
                    nc.tensor.matmul(out, wl, it3[:, k, 0:B],
                                     start=False, stop=False,
                                     skip_group_check=True)
                    nc.tensor.matmul(out, wh, it3[:, k, B:2 * B],
                                     start=False, stop=(k == KC0 - 1),
                                     skip_group_check=True)
            # one whole-tile copy per group (tile-granular deps); the upper
            # group's copy is deferred into the loop emission so it does not
            # head-of-line-block the early LIF/sig work
            def _emit_copies(g=g):
                if g == 0:
                    nc.vector.tensor_copy(H0Th[g][:], phs[g][:])
                else:
                    # upper half: ACT copy so it overlaps the DVE's phase-A
                    # LIF stream instead of serializing into it
                    nc.scalar.copy(H0Th[g][:], phs[g][:])
            if g == 0:
                _emit_copies()
            else:
                h0t_upper_copies.append(_emit_copies)

        # W1 / W2 DMAs queue after W0 (needed only once the loop's matmuls start)
        W1sb = const_pool.tile([128, NPAIR * 2 * 2 * SH], f8, tag="W1sb")
        nc.sync.dma_start(W1sb[:].rearrange("p (k c) -> p k c", k=2 * NPAIR),
                          d_W1.ap().rearrange("(k p) c -> p k c", p=128))
        w1v = W1sb[:].rearrange("p (k c) -> p k c", k=2 * NPAIR)
        W2sb = const_pool.tile([128, (SH // 128) * F3], f32r, tag="W2sb")
        nc.sync.dma_start(W2sb[:].rearrange("p (k o) -> p k o", k=SH // 128),
                          d_W2.ap().rearrange("(k p) o -> p k o", p=128))

        # ---- the 50-step recurrence -------------------------------------------
        # Emission-order design (the tile scheduler pins the simulated
        # overlap via per-engine counting semaphores, so per-engine queue
        # order is what determines pipelining):
        #   DVE: LIF0 slab(s) of step t, then the LAGGED layer-1 ops of step
        #        t-1 (fused LIF1 reading S1 from PSUM, then spk1 via a 2x-rate
        #        tensor_scalar compare) so they never block the next slab.
        #   ACT: alpha-decay prewrite for step t (ready early), then the two
        #        sig0 slabs.
        #   PE:  A-accumulation of step t-1, then the 16 DoubleRow matmuls.
        # Steps 1..PREF run slab-a work for all steps first (phase A) so DVE
        # and ACT fill the W0-DMA shadow; the H0T upper-half copies are
        # emitted between the phases.
        PREF = 10

        def emit_lif0(t, h):
            nc.vector._custom_dve(LIF, out=MPh[h][t % NB][:],
                                  in0=MPh[h][(t - 1) % NB][:],
                                  in1=H0Th[h][:], s0=(BETA if t > 1 else 0.0))

        def emit_sig0(t, h):
            nc.scalar.activation(NS0h[h][t % NB][:], MPh[h][t % NB][:],
                                 AF.Sigmoid, bias=bnbig[:], scale=BIG)

        def emit_lif1(t):
            # M1P(t) = beta * (M1P(t-1) * (M1P(t-1) <= 1)) + S1(t)  [S1 in PSUM]
            nc.vector._custom_dve(LIF, out=M1P[t % NB][:], in0=M1P[(t - 1) % NB][:],
                                  in1=ph1[t % 2][:], s0=(BETA if t > 1 else 0.0))

        SPL = 128   # spk1 threshold split: DVE gets SPL cols, ACT the rest
        def emit_sig1(t):
            # split threshold: DVE at 2x tensor_scalar rate, remainder on ACT
            nc.vector.tensor_scalar(NS1[t % NB][:, 0:SPL], M1P[t % NB][:, 0:SPL],
                                    1.0, None, AL.is_gt)
            nc.scalar.activation(NS1[t % NB][:, SPL:SH], M1P[t % NB][:, SPL:SH],
                                 AF.Sigmoid, bias=bnbig[:], scale=BIG)

        def emit_amm(t):
            nc.tensor.matmul(A_ps[:], ctis[:, (t - 1) * 128:t * 128],
                             NS1[t % NB][:],
                             start=(t == 1), stop=(t == T), skip_group_check=True)

        def emit_acopy(t):
            # alpha * S1(t-1) into step t's psum bank; matmuls accumulate on top
            nc.scalar.activation(ph1[t % 2][:], ph1[1 - (t % 2)][:], AF.Copy,
                                 scale=ALPHA)

        def emit_mms(t, h):
            ns0v = NS0h[h][t % NB][:].rearrange("p (k b) -> p k b", k=NPAIR)
            base = h * (NPAIR // 2)
            for term in range(2):
                for pi in range(NPAIR // 2):
                    lhs = ns0v[:, 2 * pi:2 * pi + 2, :]
                    gp = base + pi
                    rhs = w1v[:, 2 * gp:2 * gp + 2, term * SH:(term + 1) * SH]
                    first = (h == 0 and term == 0 and pi == 0)
                    last = (h == 1 and term == 1 and pi == NPAIR // 2 - 1)
                    nc.tensor.matmul(ph1[t % 2][:], lhs, rhs,
                                     start=(first and t == 1), stop=last,
                                     perf_mode=PM_DR, skip_group_check=True)

        # phase A: half-a LIF + sig0 for the first PREF steps (lower H0T half)
        for t in range(1, PREF + 1):
            emit_lif0(t, 0)
            emit_sig0(t, 0)
        # upper-half H0T copy lands here in the DVE queue
        for fn in h0t_upper_copies:
            fn()
        # phase B: half-b + layer-1 for the first PREF steps
        for t in range(1, PREF + 1):
            emit_lif0(t, 1)
            if t > 1:
                emit_lif1(t - 1)
            emit_sig0(t, 1)
            if t > 1:
                emit_acopy(t)
                emit_sig1(t - 1)
                emit_amm(t - 1)
            emit_mms(t, 0)
            emit_mms(t, 1)
        # steady steps
        for t in range(PREF + 1, T + 1):
            emit_lif0(t, 0)
            emit_lif0(t, 1)
            emit_lif1(t - 1)
            emit_sig0(t, 0)
            emit_acopy(t)
            emit_sig1(t - 1)
            emit_sig0(t, 1)
            emit_amm(t - 1)
            emit_mms(t, 0)
            emit_mms(t, 1)

        emit_lif1(T)
        emit_sig1(T)
        emit_amm(T)

        # ---- final: OUT_partial = A @ W2s -------------------------------------
        A_sb = out_pool.tile([128, SH], f32, tag="Asb")
        nc.vector.tensor_copy(A_sb[:], A_ps[:])
        AT = out_pool.tile([128, (SH // 128) * 128], f32r, tag="AT")
        for j in range(SH // 128):
            tp = psT_pool.tile([128, 128], f32, tag="psT", name=f"psT{j}")
            nc.tensor.transpose(tp[:], A_sb[:, j * 128:(j + 1) * 128], eye[:])
            nc.vector.tensor_copy(AT[:, j * 128:(j + 1) * 128], tp[:])
        pout = psH_pool.tile([128, F3], f32, tag="psH", name="pout")
        for j in range(SH // 128):
            nc.tensor.matmul(pout[:], AT[:, j * 128:(j + 1) * 128],
                             W2sb[:, j * F3:(j + 1) * F3],
                             start=(j == 0), stop=(j == SH // 128 - 1))
        outsb = out_pool.tile([128, F3], f32, tag="outsb")
        nc.scalar.copy(outsb[:], pout[:])
        nc.sync.dma_start(d_out.ap(), outsb[:])

    nc.compile()
    return nc


_NC_CACHE = []


def _get_nc():
    if not _NC_CACHE:
        _NC_CACHE.append(_build())
    return _NC_CACHE[0]


def kernel(inputs, W0, W1, W2):
    inputs = np.asarray(inputs, dtype=np.float32)
    W0 = np.asarray(W0, dtype=np.float32)
    W1 = np.asarray(W1, dtype=np.float32)
    W2 = np.asarray(W2, dtype=np.float32)

    nc = _get_nc()

    inT = np.ascontiguousarray(inputs.T)
    iThi = inT.astype(np.float16)
    iTlo = (inT - iThi.astype(np.float32)).astype(np.float16)
    IT = np.ascontiguousarray(np.concatenate([iThi, iTlo], axis=1))
    W0H = W0.astype(np.float16)
    W0L = np.ascontiguousarray((W0 - W0H.astype(np.float32)).astype(ml_dtypes.bfloat16))
    W0H = np.ascontiguousarray(W0H)
    eye = np.eye(128, dtype=np.float32)

    in_maps = []
    for c in range(N_CORES):
        W1s = W1[:, c * SH:(c + 1) * SH]
        W1a = W1s.astype(ml_dtypes.float8_e4m3fn)
        W1b = (W1s - W1a.astype(np.float32)).astype(ml_dtypes.float8_e4m3fn)
        W1ab = np.ascontiguousarray(np.concatenate([W1a, W1b], axis=1))
        in_maps.append({
            "IT": IT,
            "W0H": W0H,
            "W0L": W0L,
            "W1AB": W1ab,
            "W2S": np.ascontiguousarray(W2[c * SH:(c + 1) * SH, :]),
            "EYE": eye,
        })
    try:
        res = bass_utils.run_bass_kernel_spmd(nc, in_maps,
                                              core_ids=list(range(N_CORES)))
    except Exception:
        res = bass_utils.run_bass_kernel_spmd(nc, in_maps,
                                              core_ids=list(range(N_CORES)))
    out = np.zeros((B, F3), dtype=np.float32)
    for c in range(N_CORES):
        out += res.results[c]["OUT"]
    return out


# revision 53
# speedup vs baseline: 1.0564x; 1.0564x over previous
"""Trainium2 Bass kernel: 3-layer spiking NN (DSNN) forward, 50 timesteps.

Strategy (8 NeuronCores, no inter-core communication):
  - Layer-1 output features sharded 8x (256 per core); each core runs the
    full layer-0 dynamics locally (spk0 is a dense input to every shard's
    matmul) and the host sums the K-sharded final layer-2 partials.
  - H0 = inputs @ W0 (time-invariant layer-0 drive) is computed near-exactly
    via three matmul chains (ihi@Whi + ihi@Wlo + ilo@Whi with fp16 hi and
    fp16/bf16 lo splits; ~1e-4 max abs error on H0), directly in
    [feature, batch] layout (W0 chunk as the stationary operand), streamed
    in 2 column groups so the loop's lower-half work starts while the
    upper half is still in DMA (phase-A prefill of the first PREF steps).
  - Layer-0 recurrence on the pre-reset membrane MP via the fused DVE op
    DSNN_LIF_STEP: MP' = beta*(MP*(MP<=1)) + H0.  Spikes via ACT sigmoid
    saturation, written as exact {0,1} float8e4.  All concurrently-used
    regions are separate tiles (dependency tracking is tile-granular).
  - Layer-1 drive: psum += spk0 @ W1 as fp8 DoubleRow matmuls (K=256 per
    matmul, 0.5 cyc/row) over a 2-term e4m3 split of W1 (quantization error
    ~0.16%; spikes are exact in fp8 so only W1 rounding enters).
  - Layer-1 synaptic state S1 lives entirely in PSUM: ACT performs the
    alpha-decay as a scaled psum->psum copy into the next ping-pong bank
    and the matmul group accumulates on top (start=False; note start=True
    zeroes the WHOLE psum bank, so it is only used for the first group).
    The fused LIF op reads S1 straight from PSUM for the membrane update;
    spk1 threshold is split across DVE (2x-rate tensor_scalar) and ACT.
  - Output accumulation A = sum_t c_t * spk1(t) on the PE via c_t-scaled
    identity stationaries; final A @ W2 shard as 2 fp32r matmuls; host sums
    the 8 partial outputs.
Measured (TimelineSim): 181103 ns vs 219440 ns baseline; rel err 1.66e-2.
"""

import numpy as np
import ml_dtypes
from contextlib import ExitStack

import concourse.bacc as bacc
import concourse.bass as bass
import concourse.mybir as mybir
import concourse.tile as tile
from concourse import bass_utils
from concourse import dve_ops as _DOPS
from concourse.dve_spec import Spec, Src0, Src1, C0, One, lower as _dve_lower
from concourse.dve_uop import DveOpSpec as _DveOpSpec

ALPHA = 0.9
BETA = 0.85
T = 50
B = 128                # batch
F0, F1, F3 = 1024, 2048, 512
N_CORES = 8
SH = F1 // N_CORES     # 256 layer-1 features per core
KC0 = F0 // 128        # 8 contraction chunks for H0
NPAIR = F1 // 256      # 8 DoubleRow contraction pairs for layer-1
NG = 2                 # H0 column groups (feature groups of 1024)
GW = F1 // NG          # 1024 features per group
BIG = 1.0e30

f32 = mybir.dt.float32
f32r = mybir.dt.float32r
f16 = mybir.dt.float16
bf16 = mybir.dt.bfloat16
f8 = mybir.dt.float8e4
AL = mybir.AluOpType
AF = mybir.ActivationFunctionType
PM_DR = mybir.MatmulPerfMode.DoubleRow

LIF_SLABS = 2          # layer-0 LIF slabs (1024 cols each)
SIG2_STEPS = 8         # steps that use 2 spike-extraction slabs (prologue overlap)


def _register_lif():
    """Fused LIF step on the pre-reset membrane:
    out = s0 * (in0 * (in0 <= 1)) + in1  (reset-gate, leak, drive)."""
    name = "DSNN_LIF_STEP"
    for op in _DOPS.OPS:
        if op.name == name:
            return op
    body = (Src0 * (Src0 <= One)) * C0 + Src1
    spec = Spec(body=body,
                reference=lambda in0, in1, s0, s1, imm2:
                    ((in0 * (in0 <= 1.0)) * np.float32(s0) + in1).astype(np.float32))
    row = max(_DOPS._SUB_OPCODE_FOR_NAME.values()) + 1
    _DOPS._SUB_OPCODE_FOR_NAME[name] = row
    shas = {}
    for ver in ("v3", "v4"):
        uops = _dve_lower(spec, ver=ver)
        shas[ver] = _DveOpSpec(name=name, opcode=row, uops=uops, rd1_en=True).sha(ver)
    op = _DOPS.DveOp(name, spec, subdim=False, uops_sha=shas)
    _DOPS.OPS.append(op)
    _DOPS.CUSTOM_DVE_SPECS[name] = spec
    return op


LIF = _register_lif()


def _coeffs():
    # m2(T) = sum_{t=1..T} c[t-1] * h2(t)
    c = np.zeros(T, dtype=np.float64)
    for s in range(T):
        tau = s + 1
        c[s] = sum(BETA ** (T - t) * ALPHA ** (t - tau) for t in range(tau, T + 1))
    return c.astype(np.float32)


def _build():
    nc = bacc.Bacc("TRN2", target_bir_lowering=False, debug=False)
    d_IT = nc.dram_tensor("IT", [F0, 2 * B], f16, kind="ExternalInput")     # [infeat, hi|lo batch]
    d_W0H = nc.dram_tensor("W0H", [F0, F1], f16, kind="ExternalInput")
    d_W0L = nc.dram_tensor("W0L", [F0, F1], bf16, kind="ExternalInput")
    d_W1 = nc.dram_tensor("W1AB", [F1, 2 * SH], f8, kind="ExternalInput")   # [feat, A|B cols]
    d_W2 = nc.dram_tensor("W2S", [SH, F3], f32r, kind="ExternalInput")
    d_eye = nc.dram_tensor("EYE", [128, 128], f32, kind="ExternalInput")
    d_out = nc.dram_tensor("OUT", [B, F3], f32, kind="ExternalOutput")

    coefs = _coeffs()

    with tile.TileContext(nc) as tc, ExitStack() as ctx:
        const_pool = ctx.enter_context(tc.tile_pool(name="const", bufs=1))
        state_pool = ctx.enter_context(tc.tile_pool(name="state", bufs=1))
        w0h_pool = ctx.enter_context(tc.tile_pool(name="w0h", bufs=5))
        w0l_pool = ctx.enter_context(tc.tile_pool(name="w0l", bufs=5))
        out_pool = ctx.enter_context(tc.tile_pool(name="outp", bufs=1))
        psH_pool = ctx.enter_context(tc.tile_pool(name="psH", bufs=1, space="PSUM"))
        ps1_pool = ctx.enter_context(tc.tile_pool(name="ps1", bufs=3, space="PSUM"))
        psA_pool = ctx.enter_context(tc.tile_pool(name="psA", bufs=1, space="PSUM"))
        psT_pool = ctx.enter_context(tc.tile_pool(name="psT", bufs=1, space="PSUM"))

        # ---- resident constants (DMAed first; small) --------------------------
        eye = const_pool.tile([128, 128], f32, tag="eye")
        nc.sync.dma_start(eye[:], d_eye.ap())
        itb = const_pool.tile([128, KC0 * 2 * B], f16, tag="itb")
        nc.sync.dma_start(itb[:].rearrange("p (k c) -> p k c", k=KC0),
                          d_IT.ap().rearrange("(k p) c -> p k c", p=128))
        it3 = itb[:].rearrange("p (k c) -> p k c", k=KC0)

        bnbig = const_pool.tile([128, 1], f32, tag="bnbig")
        nc.vector.memset(bnbig[:], -BIG)

        # PE warm-up: the p-state ramp needs ~3us of continuous busy before
        # the PE hits full speed.  A run of dummy eye transposes bridges the
        # gap until the first W0 chunk lands so the H0 matmuls start the
        # stream fully ramped instead of at half rate.
        warm = psT_pool.tile([128, 128], f32, tag="psT", name="warm")
        for _ in range(16):
            nc.tensor.transpose(warm[:], eye[:], eye[:])

        # c_t-scaled identity tiles for the A accumulation (DVE 2x tensor_scalar)
        ctis = const_pool.tile([128, T * 128], f32r, tag="ctis")
        for t in range(T):
            nc.vector.tensor_scalar(ctis[:, t * 128:(t + 1) * 128], eye[:],
                                    float(coefs[t]), None, AL.mult)

        # ---- state ------------------------------------------------------------
        # Dependency tracking is TILE-granular: every region used
        # concurrently by different engines must be its own tile.  Layer-0
        # state is split into column halves (a = features 0:1024,
        # b = 1024:2048) and rotated over NB buffers.
        NB = 3   # state-buffer rotation depth (>2 gives the scheduler slack)
        HW2 = F1 // 2
        H0Th = [state_pool.tile([128, HW2], f32, tag=f"H0T{h}", name=f"H0T{h}")
                for h in range(2)]                             # [feat, batch] drive
        MPh = [[state_pool.tile([128, HW2], f32, tag=f"MP{h}{i}", name=f"MP{h}{i}")
                for i in range(NB)] for h in range(2)]         # layer-0 membrane
        NS0h = [[state_pool.tile([128, HW2], f8, tag=f"NS0{h}{i}", name=f"NS0{h}{i}")
                 for i in range(NB)] for h in range(2)]        # spk0 {0,1} fp8
        M1P = [state_pool.tile([128, SH], f32, tag=f"M1P{i}", name=f"M1P{i}")
               for i in range(NB)]                             # layer-1 membrane (pre-reset)
        NS1 = [state_pool.tile([128, SH], f32r, tag=f"NS1{i}", name=f"NS1{i}")
               for i in range(NB)]
        nc.scalar.memzero(MPh[0][0][:])
        nc.scalar.memzero(MPh[1][0][:])
        nc.scalar.memzero(M1P[0][:])

        # layer-1 synaptic state: PSUM ping-pong banks (S1' = alpha*S1 + spk0@W1)
        ph1 = [ps1_pool.tile([128, SH], f32, tag="ps1", name=f"ph1{i}")
               for i in range(3)]
        A_ps = psA_pool.tile([128, SH], f32, tag="psA")

        # ---- H0 = (inputs @ W0)^T via 3 fp16 chains, streamed in NG groups ----
        phs = [psH_pool.tile([128, GW], f32, tag="psH", name=f"psH{g}")
               for g in range(NG)]
        h0t_upper_copies = []
        for g in range(NG):
            cols = slice(g * GW, (g + 1) * GW)
            for k in range(KC0):
                th = w0h_pool.tile([128, GW], f16, tag="w0h", name=f"w0h_{g}_{k}")
                nc.sync.dma_start(th[:], d_W0H.ap()[k * 128:(k + 1) * 128, cols])
                tl = w0l_pool.tile([128, GW], bf16, tag="w0l", name=f"w0l_{g}_{k}")
                nc.sync.dma_start(tl[:], d_W0L.ap()[k * 128:(k + 1) * 128, cols])
                for jj in range(GW // 128):
                    out = phs[g][:, jj * 128:(jj + 1) * 128]
                    wh = th[:, jj * 128:(jj + 1) * 128]
                    wl = tl[:, jj * 128:(jj + 1) * 128]
                    # start=True zeroes the WHOLE psum bank: only the first
                    # 128-slice of each 512-col bank may set it
                    nc.tensor.matmul(out, wh, it3[:, k, 0:B],
                                     start=(k == 0 and jj % 4 == 0), stop=False,
                                     skip_group_check=True)
                    nc.tensor.matmul(out, wl, it3[:, k, 0:B],
                                     start=False, stop=False,
                                     skip_group_check=True)
                    nc.tensor.matmul(out, wh, it3[:, k, B:2 * B],
                                     start=False, stop=(k == KC0 - 1),
                                     skip_group_check=True)
            # one whole-tile copy per group (tile-granular deps); the upper
            # group's copy is deferred into the loop emission so it does not
            # head-of-line-block the early LIF/sig work
            def _emit_copies(g=g):
                if g == 0:
                    nc.vector.tensor_copy(H0Th[g][:], phs[g][:])
                else:
                    # upper half: ACT copy so it overlaps the DVE's phase-A
                    # LIF stream instead of serializing into it
                    nc.scalar.copy(H0Th[g][:], phs[g][:])
            if g == 0:
                _emit_copies()
            else:
                h0t_upper_copies.append(_emit_copies)

        # W1 / W2 DMAs queue after W0 (needed only once the loop's matmuls start)
        W1sb = const_pool.tile([128, NPAIR * 2 * 2 * SH], f8, tag="W1sb")
        nc.sync.dma_start(W1sb[:].rearrange("p (k c) -> p k c", k=2 * NPAIR),
                          d_W1.ap().rearrange("(k p) c -> p k c", p=128))
        w1v = W1sb[:].rearrange("p (k c) -> p k c", k=2 * NPAIR)
        W2sb = const_pool.tile([128, (SH // 128) * F3], f32r, tag="W2sb")
        nc.sync.dma_start(W2sb[:].rearrange("p (k o) -> p k o", k=SH // 128),
                          d_W2.ap().rearrange("(k p) o -> p k o", p=128))

        # ---- the 50-step recurrence -------------------------------------------
        # Emission-order design (the tile scheduler pins the simulated
        # overlap via per-engine counting semaphores, so per-engine queue
        # order is what determines pipelining):
        #   DVE: LIF0 slab(s) of step t, then the LAGGED layer-1 ops of step
        #        t-1 (fused LIF1 reading S1 from PSUM, then spk1 via a 2x-rate
        #        tensor_scalar compare) so they never block the next slab.
        #   ACT: alpha-decay prewrite for step t (ready early), then the two
        #        sig0 slabs.
        #   PE:  A-accumulation of step t-1, then the 16 DoubleRow matmuls.
        # Steps 1..PREF run slab-a work for all steps first (phase A) so DVE
        # and ACT fill the W0-DMA shadow; the H0T upper-half copies are
        # emitted between the phases.
        PREF = 10

        def emit_lif0(t, h):
            nc.vector._custom_dve(LIF, out=MPh[h][t % NB][:],
                                  in0=MPh[h][(t - 1) % NB][:],
                                  in1=H0Th[h][:], s0=(BETA if t > 1 else 0.0))

        def emit_sig0(t, h):
            nc.scalar.activation(NS0h[h][t % NB][:], MPh[h][t % NB][:],
                                 AF.Sigmoid, bias=bnbig[:], scale=BIG)

        def emit_lif1(t):
            # M1P(t) = beta * (M1P(t-1) * (M1P(t-1) <= 1)) + S1(t)  [S1 in PSUM]
            nc.vector._custom_dve(LIF, out=M1P[t % NB][:], in0=M1P[(t - 1) % NB][:],
                                  in1=ph1[t % 3][:], s0=(BETA if t > 1 else 0.0))

        SPL = 128   # spk1 threshold split: DVE gets SPL cols, ACT the rest
        def emit_sig1(t):
            # split threshold: DVE at 2x tensor_scalar rate, remainder on ACT
            nc.vector.tensor_scalar(NS1[t % NB][:, 0:SPL], M1P[t % NB][:, 0:SPL],
                                    1.0, None, AL.is_gt)
            nc.scalar.activation(NS1[t % NB][:, SPL:SH], M1P[t % NB][:, SPL:SH],
                                 AF.Sigmoid, bias=bnbig[:], scale=BIG)

        def emit_amm(t):
            nc.tensor.matmul(A_ps[:], ctis[:, (t - 1) * 128:t * 128],
                             NS1[t % NB][:],
                             start=(t == 1), stop=(t == T), skip_group_check=True)

        def emit_acopy(t):
            # alpha * S1(t-1) into step t's psum bank; matmuls accumulate on top
            nc.scalar.activation(ph1[t % 3][:], ph1[(t - 1) % 3][:], AF.Copy,
                                 scale=ALPHA)

        def emit_mms(t, h):
            ns0v = NS0h[h][t % NB][:].rearrange("p (k b) -> p k b", k=NPAIR)
            base = h * (NPAIR // 2)
            for term in range(2):
                for pi in range(NPAIR // 2):
                    lhs = ns0v[:, 2 * pi:2 * pi + 2, :]
                    gp = base + pi
                    rhs = w1v[:, 2 * gp:2 * gp + 2, term * SH:(term + 1) * SH]
                    first = (h == 0 and term == 0 and pi == 0)
                    last = (h == 1 and term == 1 and pi == NPAIR // 2 - 1)
                    nc.tensor.matmul(ph1[t % 3][:], lhs, rhs,
                                     start=(first and t == 1), stop=last,
                                     perf_mode=PM_DR, skip_group_check=True)

        # phase A: half-a LIF + sig0 for the first PREF steps (lower H0T half)
        for t in range(1, PREF + 1):
            emit_lif0(t, 0)
            emit_sig0(t, 0)
        # upper-half H0T copy lands here in the DVE queue
        for fn in h0t_upper_copies:
            fn()
        # phase B: half-b + layer-1 for the first PREF steps
        for t in range(1, PREF + 1):
            emit_lif0(t, 1)
            if t > 2:
                emit_lif1(t - 2)
            emit_sig0(t, 1)
            if t > 1:
                emit_acopy(t)
            if t > 2:
                emit_sig1(t - 2)
                emit_amm(t - 2)
            emit_mms(t, 0)
            emit_mms(t, 1)
        # steady steps
        for t in range(PREF + 1, T + 1):
            emit_lif0(t, 0)
            emit_lif0(t, 1)
            emit_lif1(t - 2)
            emit_sig0(t, 0)
            emit_acopy(t)
            emit_sig1(t - 2)
            emit_sig0(t, 1)
            emit_amm(t - 2)
            emit_mms(t, 0)
            emit_mms(t, 1)

        for tt in (T - 1, T):
            emit_lif1(tt)
            emit_sig1(tt)
            emit_amm(tt)

        # ---- final: OUT_partial = A @ W2s -------------------------------------
        A_sb = out_pool.tile([128, SH], f32, tag="Asb")
        nc.vector.tensor_copy(A_sb[:], A_ps[:])
        AT = out_pool.tile([128, (SH // 128) * 128], f32r, tag="AT")
        for j in range(SH // 128):
            tp = psT_pool.tile([128, 128], f32, tag="psT", name=f"psT{j}")
            nc.tensor.transpose(tp[:], A_sb[:, j * 128:(j + 1) * 128], eye[:])
            nc.vector.tensor_copy(AT[:, j * 128:(j + 1) * 128], tp[:])
        pout = psH_pool.tile([128, F3], f32, tag="psH", name="pout")
        for j in range(SH // 128):
            nc.tensor.matmul(pout[:], AT[:, j * 128:(j + 1) * 128],
                             W2sb[:, j * F3:(j + 1) * F3],
                             start=(j == 0), stop=(j == SH // 128 - 1))
        outsb = out_pool.tile([128, F3], f32, tag="outsb")
        nc.scalar.copy(outsb[:], pout[:])
        nc.sync.dma_start(d_out.ap(), outsb[:])

    nc.compile()
    return nc


_NC_CACHE = []


def _get_nc():
    if not _NC_CACHE:
        _NC_CACHE.append(_build())
    return _NC_CACHE[0]


def kernel(inputs, W0, W1, W2):
    inputs = np.asarray(inputs, dtype=np.float32)
    W0 = np.asarray(W0, dtype=np.float32)
    W1 = np.asarray(W1, dtype=np.float32)
    W2 = np.asarray(W2, dtype=np.float32)

    nc = _get_nc()

    inT = np.ascontiguousarray(inputs.T)
    iThi = inT.astype(np.float16)
    iTlo = (inT - iThi.astype(np.float32)).astype(np.float16)
    IT = np.ascontiguousarray(np.concatenate([iThi, iTlo], axis=1))
    W0H = W0.astype(np.float16)
    W0L = np.ascontiguousarray((W0 - W0H.astype(np.float32)).astype(ml_dtypes.bfloat16))
    W0H = np.ascontiguousarray(W0H)
    eye = np.eye(128, dtype=np.float32)

    in_maps = []
    for c in range(N_CORES):
        W1s = W1[:, c * SH:(c + 1) * SH]
        W1a = W1s.astype(ml_dtypes.float8_e4m3fn)
        W1b = (W1s - W1a.astype(np.float32)).astype(ml_dtypes.float8_e4m3fn)
        W1ab = np.ascontiguousarray(np.concatenate([W1a, W1b], axis=1))
        in_maps.append({
            "IT": IT,
            "W0H": W0H,
            "W0L": W0L,
            "W1AB": W1ab,
            "W2S": np.ascontiguousarray(W2[c * SH:(c + 1) * SH, :]),
            "EYE": eye,
        })
    try:
        res = bass_utils.run_bass_kernel_spmd(nc, in_maps,
                                              core_ids=list(range(N_CORES)))
    except Exception:
        res = bass_utils.run_bass_kernel_spmd(nc, in_maps,
                                              core_ids=list(range(N_CORES)))
    out = np.zeros((B, F3), dtype=np.float32)
    for c in range(N_CORES):
        out += res.results[c]["OUT"]
    return out
